# revision 1
# baseline (speedup 1.0000x reference)
"""Trainium2 Bass kernel for nn_CopyModule (pointer-generator copy head).

Full-input contract: kernel(**inputs) takes the unsharded numpy inputs and
returns the full [4, 512, 32000] f32 output. Internally shards over
(batch, T/2) across 8 NeuronCores -- fully SPMD, no collectives.

Per-core math (replicates the reference numerics, including the f32
(1 - sigmoid) cancellation and the +1e-12 epsilon):
    S[t]    = sum_v exp(x[t, v])                  (ACT exp pass w/ accum)
    A       = mean_h attn[h]                      (PE identity/16 accumulate)
    ctx     = A @ enc                             (PE)
    gate    = dls@w1 + die@w2 + ctx@w3 + biases   (DVE tensor_tensor_reduce)
    p       = 1/(1+exp(-gate)); omp = 1 - p       (ACT + DVE, f32 cancellation)
    s[t]    = omp/S ; rho[t] = p*S/omp
    c       = A @ M_onehot  (dedup per v-tile slots); cp = rho * c
    out     = Ln((e^x + cp^T-expanded-sparse) * s[t] + 1e-12)

The scatter-add into vocab positions needs no indirect DMA: unique ids are
placed into 24 slots per 500-wide v-tile (slot space 64*24 = 1536 = 12
partition chunks), so each v-tile touches a compile-time-known set of slot
chunks. The sparse contribution is expanded to dense inside PSUM with small
one-hot matmuls (one-hots built on device via iota/is_equal), and the ACT Ln
pass fuses the (1-p)/S scale and +eps. e^x stays resident in SBUF as fp16,
one 128-row chunk at a time.
"""

import numpy as np

# ---- problem constants (hardcoded per contract) ----
B, H, T, S, D, V = 4, 16, 512, 512, 1024, 32000
EPS = 1e-12
TH = 256          # rows per core
P = 128           # partitions
NCH = TH // P     # 2 t-chunks per core
W1 = 2000         # pass-1 (exp) tile width
NT1 = V // W1     # 32
W2 = 500          # pass-2 (Ln) tile width == one PSUM bank
NT2 = V // W2     # 64
CAP = 24          # unique-id slots per v-tile
NSLOT = NT2 * CAP # 1536
NJC = NSLOT // P  # 12 slot chunks

_CACHE = {}


def _build_nc():
    import concourse.bass as bass
    import concourse.bacc as bacc
    import concourse.mybir as mybir
    import concourse.tile as tile
    from contextlib import ExitStack

    f32 = mybir.dt.float32
    f16 = mybir.dt.float16
    Af = mybir.ActivationFunctionType
    Op = mybir.AluOpType
    Ax = mybir.AxisListType

    nc = bacc.Bacc()

    x_d = nc.dram_tensor("x", [TH, V], f32, kind="ExternalInput")
    attn_d = nc.dram_tensor("attn", [H, TH, S], f32, kind="ExternalInput")
    enc_d = nc.dram_tensor("enc", [P, 4, D], f32, kind="ExternalInput")
    dls_d = nc.dram_tensor("dls", [TH, D], f32, kind="ExternalInput")
    die_d = nc.dram_tensor("die", [TH, D], f32, kind="ExternalInput")
    wrep_d = nc.dram_tensor("wrep", [P, 3 * D], f32, kind="ExternalInput")
    ids_d = nc.dram_tensor("idsf", [P, 4], f32, kind="ExternalInput")
    colsr_d = nc.dram_tensor("colsr", [P, NSLOT], f32, kind="ExternalInput")
    shift_d = nc.dram_tensor("shift", [P, NJC, NT2], f32, kind="ExternalInput")
    bias_d = nc.dram_tensor("biasr", [P, 1], f32, kind="ExternalInput")
    identf_d = nc.dram_tensor("identf", [P, P], f32, kind="ExternalInput")
    identh_d = nc.dram_tensor("identh", [P, P], f16, kind="ExternalInput")
    idiv_d = nc.dram_tensor("idiv16", [P, P], f32, kind="ExternalInput")
    out_d = nc.dram_tensor("out", [TH, V], f32, kind="ExternalOutput")

    with tile.TileContext(nc) as tc, ExitStack() as ctx:
        # ---- long-lived pools ----
        const = ctx.enter_context(tc.tile_pool(name="const", bufs=1))
        work = ctx.enter_context(tc.tile_pool(name="work", bufs=1))
        xin_p = ctx.enter_context(tc.tile_pool(name="xin", bufs=3))
        ex_p = ctx.enter_context(tc.tile_pool(name="ex", bufs=1))
        ps_t = ctx.enter_context(tc.tile_pool(name="pst", bufs=2, space="PSUM"))
        ps_c = ctx.enter_context(tc.tile_pool(name="psc", bufs=1, space="PSUM"))

        identf = const.tile([P, P], f32)
        nc.sync.dma_start(identf[:], identf_d[:])
        identh = const.tile([P, P], f16)
        nc.sync.dma_start(identh[:], identh_d[:])
        iota = const.tile([P, W2], f32)
        nc.gpsimd.iota(iota[:], pattern=[[1, W2]], base=0, channel_multiplier=0,
                       allow_small_or_imprecise_dtypes=True)
        eps_t = const.tile([P, 1], f32)
        nc.vector.memset(eps_t[:], EPS)
        shift_sb = const.tile([P, NJC, NT2], f32)
        nc.sync.dma_start(shift_sb[:], shift_d[:])
        m_sb = const.tile([P, 4, NSLOT], f16)
        cpt = [const.tile([P, TH], f32, tag=f"cpt{jc}", name=f"cpt{jc}")
               for jc in range(NJC)]
        at_f32 = [const.tile([P, TH], f32, tag=f"atf{sc}", name=f"atf{sc}")
                  for sc in range(4)]
        at_f16 = [const.tile([P, TH], f16, tag=f"ath{sc}", name=f"ath{sc}")
                  for sc in range(4)]
        gate = [const.tile([P, 1], f32, tag=f"g3_{tcn}", name=f"g3_{tcn}")
                for tcn in range(NCH)]

        # ---- phase B (transient pools, released before pass 2) ----
        with tc.tile_pool(name="pb", bufs=1) as pb, \
             tc.tile_pool(name="abp", bufs=4) as ab_p, \
             tc.tile_pool(name="psa", bufs=1, space="PSUM") as ps_a, \
             tc.tile_pool(name="psctx", bufs=1, space="PSUM") as ps_ctx:
            idiv16 = pb.tile([P, P], f32)
            nc.sync.dma_start(idiv16[:], idiv_d[:])
            bias_t = pb.tile([P, 1], f32)
            nc.sync.dma_start(bias_t[:], bias_d[:])
            wrep = pb.tile([P, 3 * D], f32)
            nc.sync.dma_start(wrep[:], wrep_d[:])
            cols_rep = pb.tile([P, NSLOT], f32)
            nc.sync.dma_start(cols_rep[:], colsr_d[:])
            ids_sb = pb.tile([P, 4], f32)
            nc.sync.dma_start(ids_sb[:], ids_d[:])
            enc_sb = pb.tile([P, 4, D], f32)
            nc.sync.dma_start(enc_sb[:], enc_d[:])

            # M one-hot [s-chunk][128, NSLOT] f16: M[s, j] = (cols[j] == ids[s])
            for sc in range(4):
                nc.vector.tensor_scalar(m_sb[:, sc, :], cols_rep[:],
                                        ids_sb[:, sc:sc + 1], None, op0=Op.is_equal)

            # A = mean_h attn, then A^T per s-chunk
            for tcn in range(NCH):
                pa = ps_a.tile([P, S], f32, tag="psA", name=f"psA{tcn}")
                for h in range(H):
                    abt = ab_p.tile([P, S], f32, tag="abt", name=f"abt{tcn}_{h}")
                    nc.sync.dma_start(abt[:], attn_d[h, tcn * P:(tcn + 1) * P, :])
                    nc.tensor.matmul(pa[:], lhsT=idiv16[:], rhs=abt[:],
                                     start=(h == 0), stop=(h == H - 1))
                a_t = pb.tile([P, S], f32, tag="asb", bufs=2, name=f"asb{tcn}")
                nc.vector.tensor_copy(a_t[:], pa[:])
                for sc in range(4):
                    pt = ps_t.tile([P, P], f32, tag="pstT", name=f"pstA{tcn}_{sc}")
                    nc.tensor.transpose(pt[:], a_t[:, sc * P:(sc + 1) * P], identf[:])
                    nc.vector.tensor_copy(at_f32[sc][:, tcn * P:(tcn + 1) * P], pt[:])
                    nc.vector.tensor_copy(at_f16[sc][:, tcn * P:(tcn + 1) * P], pt[:])

            # ctx and gate
            for tcn in range(NCH):
                pctx = ps_ctx.tile([P, D], f32, tag="psctx", name=f"psctx{tcn}")
                for dh in range(2):
                    for sc in range(4):
                        nc.tensor.matmul(pctx[:, dh * 512:(dh + 1) * 512],
                                         lhsT=at_f32[sc][:, tcn * P:(tcn + 1) * P],
                                         rhs=enc_sb[:, sc, dh * 512:(dh + 1) * 512],
                                         start=(sc == 0), stop=(sc == 3))
                dls_t = pb.tile([P, D], f32, tag="hid", bufs=2, name=f"dls{tcn}")
                nc.sync.dma_start(dls_t[:], dls_d[tcn * P:(tcn + 1) * P, :])
                die_t = pb.tile([P, D], f32, tag="hid", bufs=2, name=f"die{tcn}")
                nc.sync.dma_start(die_t[:], die_d[tcn * P:(tcn + 1) * P, :])
                trash = pb.tile([P, D], f32, tag="ttrtrash", name=f"tt{tcn}")
                g1 = work.tile([P, 1], f32, tag=f"g1_{tcn}", name=f"g1_{tcn}")
                g2 = work.tile([P, 1], f32, tag=f"g2_{tcn}", name=f"g2_{tcn}")
                g3 = work.tile([P, 1], f32, tag=f"g3p_{tcn}", name=f"g3p_{tcn}")
                nc.vector.tensor_tensor(trash[:], dls_t[:], wrep[:, 0:D], op=Op.mult)
                nc.vector.tensor_reduce(g1[:], trash[:], axis=Ax.X, op=Op.add)
                nc.vector.tensor_tensor(trash[:], die_t[:], wrep[:, D:2 * D], op=Op.mult)
                nc.vector.tensor_reduce(g2[:], trash[:], axis=Ax.X, op=Op.add)
                nc.vector.tensor_tensor(trash[:], pctx[:], wrep[:, 2 * D:3 * D], op=Op.mult)
                nc.vector.tensor_reduce(g3[:], trash[:], axis=Ax.X, op=Op.add)
                nc.vector.tensor_tensor(g1[:], g1[:], g2[:], op=Op.add)
                nc.vector.tensor_tensor(g3[:], g3[:], bias_t[:], op=Op.add)
                nc.vector.tensor_tensor(gate[tcn][:], g1[:], g3[:], op=Op.add)

        # ---- pass2-era pools (reuse released phase-B space) ----
        e_p = ctx.enter_context(tc.tile_pool(name="eoh", bufs=4))
        out_p = ctx.enter_context(tc.tile_pool(name="outp", bufs=3))
        ps_2 = ctx.enter_context(tc.tile_pool(name="ps2", bufs=5, space="PSUM"))

        for tcn in range(NCH):
            # pass 1: exp + accumulated row sums; e^x resident as f16
            ex_t = ex_p.tile([P, V], f16, tag="ex", name=f"ex{tcn}")
            sacc = work.tile([P, NT1], f32, tag="sacc", bufs=2, name=f"sacc{tcn}")
            for i in range(NT1):
                xt = xin_p.tile([P, W1], f32, tag="xin", name=f"x{tcn}_{i}")
                nc.sync.dma_start(xt[:], x_d[tcn * P:(tcn + 1) * P, i * W1:(i + 1) * W1])
                nc.scalar.activation(ex_t[:, i * W1:(i + 1) * W1], xt[:], Af.Exp,
                                     accum_out=sacc[:, i:i + 1])
            s_sum = work.tile([P, 1], f32, tag=f"ssum{tcn}", name=f"ssum{tcn}")
            nc.vector.tensor_reduce(s_sum[:], sacc[:], axis=Ax.X, op=Op.add)

            # scalar plumbing (all [128, 1] f32)
            u_t = work.tile([P, 1], f32, tag=f"u{tcn}", name=f"u{tcn}")
            nc.scalar.activation(u_t[:], gate[tcn][:], Af.Exp, scale=-1.0)
            w1p = work.tile([P, 1], f32, tag=f"w1p{tcn}", name=f"w1p{tcn}")
            nc.vector.tensor_scalar_add(w1p[:], u_t[:], 1.0)
            p_t = work.tile([P, 1], f32, tag=f"p{tcn}", name=f"p{tcn}")
            nc.vector.reciprocal(p_t[:], w1p[:])
            omp = work.tile([P, 1], f32, tag=f"omp{tcn}", name=f"omp{tcn}")
            nc.vector.tensor_scalar(omp[:], p_t[:], -1.0, 1.0, op0=Op.mult, op1=Op.add)
            sinv = work.tile([P, 1], f32, tag=f"sinv{tcn}", name=f"sinv{tcn}")
            nc.vector.reciprocal(sinv[:], s_sum[:])
            s_t = work.tile([P, 1], f32, tag=f"s{tcn}", name=f"s{tcn}")
            nc.vector.tensor_tensor(s_t[:], omp[:], sinv[:], op=Op.mult)
            ps_f = work.tile([P, 1], f32, tag=f"pS{tcn}", name=f"pS{tcn}")
            nc.vector.tensor_tensor(ps_f[:], p_t[:], s_sum[:], op=Op.mult)
            oinv = work.tile([P, 1], f32, tag=f"oinv{tcn}", name=f"oinv{tcn}")
            nc.vector.reciprocal(oinv[:], omp[:])
            rho = work.tile([P, 1], f32, tag=f"rho{tcn}", name=f"rho{tcn}")
            nc.vector.tensor_tensor(rho[:], ps_f[:], oinv[:], op=Op.mult)

            # c' = rho * (A @ M) in slot space, transposed to [slot, t]
            cp_sb = work.tile([P, NSLOT], f32, tag="cpsb", bufs=2, name=f"cp{tcn}")
            for sec in range(3):
                pc = ps_c.tile([P, 512], f32, tag="psc", name=f"psc{tcn}_{sec}")
                for sc in range(4):
                    nc.tensor.matmul(pc[:],
                                     lhsT=at_f16[sc][:, tcn * P:(tcn + 1) * P],
                                     rhs=m_sb[:, sc, sec * 512:(sec + 1) * 512],
                                     start=(sc == 0), stop=(sc == 3))
                nc.vector.tensor_scalar_mul(cp_sb[:, sec * 512:(sec + 1) * 512],
                                            pc[:], rho[:, :1])
            for jc in range(NJC):
                pt = ps_t.tile([P, P], f32, tag="pstT", name=f"pstC{tcn}_{jc}")
                nc.tensor.transpose(pt[:], cp_sb[:, jc * P:(jc + 1) * P], identf[:])
                nc.vector.tensor_copy(cpt[jc][:, tcn * P:(tcn + 1) * P], pt[:])

            # pass 2: psum = e^x + sparse ; out = Ln(psum * s + eps)
            for i in range(NT2):
                jset = [(CAP * i) // P]
                if (CAP * i + CAP - 1) // P != jset[0]:
                    jset.append(jset[0] + 1)
                p2 = ps_2.tile([P, W2], f32, tag="ps2", name=f"p2_{tcn}_{i}")
                nc.tensor.matmul(p2[:], lhsT=identh[:],
                                 rhs=ex_t[:, i * W2:(i + 1) * W2],
                                 start=True, stop=False)
                for k, jc in enumerate(jset):
                    et = e_p.tile([P, W2], f32, tag="eoh", name=f"e{tcn}_{i}_{k}")
                    eng = nc.vector if ((i + k) % 2 == 0) else nc.gpsimd
                    eng.tensor_scalar(et[:], iota[:], shift_sb[:, jc, i:i + 1],
                                      None, op0=Op.is_equal)
                    nc.tensor.matmul(p2[:], lhsT=cpt[jc][:, tcn * P:(tcn + 1) * P],
                                     rhs=et[:], start=False, stop=(k == len(jset) - 1))
                if i % 4 == 0:
                    o_t = out_p.tile([P, 4 * W2], f32, tag="outp", name=f"o{tcn}_{i}")
                nc.scalar.activation(o_t[:, (i % 4) * W2:(i % 4 + 1) * W2], p2[:],
                                     Af.Ln, bias=eps_t[:, :1], scale=s_t[:, :1])
                if i % 4 == 3:
                    nc.scalar.dma_start(
                        out_d[tcn * P:(tcn + 1) * P, (i - 3) * W2:(i + 1) * W2],
                        o_t[:])

    nc.finalize()
    return nc


def _get_nc():
    if "nc" not in _CACHE:
        _CACHE["nc"] = _build_nc()
    return _CACHE["nc"]


def _prep_core_inputs(inputs, b, th):
    t0 = th * TH
    x = np.ascontiguousarray(np.asarray(inputs["logits"], np.float32)[b, t0:t0 + TH])
    attn = np.ascontiguousarray(
        np.asarray(inputs["decoder_attention"], np.float32)[b, :, t0:t0 + TH, :])
    enc = np.ascontiguousarray(
        np.asarray(inputs["encoder_last_hidden_state"], np.float32)[b]
        .reshape(4, P, D).transpose(1, 0, 2))
    dls = np.ascontiguousarray(np.asarray(inputs["decoder_last_hidden_state"], np.float32)[b, t0:t0 + TH])
    die = np.ascontiguousarray(np.asarray(inputs["decoder_input_embeds"], np.float32)[b, t0:t0 + TH])
    wcat = np.concatenate([np.asarray(inputs["w_logits"], np.float32),
                           np.asarray(inputs["w_embeds"], np.float32),
                           np.asarray(inputs["w_enc"], np.float32)])
    ids = np.asarray(inputs["enc_input_ids"]).astype(np.int64)[b]
    bias_total = (float(np.asarray(inputs["b_logits"])) + float(np.asarray(inputs["b_embeds"]))
                  + float(np.asarray(inputs["b_enc"])) + float(np.asarray(inputs["bias"])))

    cols = np.full(NSLOT, -1.0, np.float32)
    for i in range(NT2):
        u = np.unique(ids[(ids >= W2 * i) & (ids < W2 * (i + 1))])
        if len(u) > CAP:
            raise ValueError(f"v-tile {i} has {len(u)} unique ids > CAP={CAP}")
        cols[CAP * i:CAP * i + len(u)] = u.astype(np.float32)
    shift = (cols[:, None] - (W2 * np.arange(NT2, dtype=np.float32))[None, :]).astype(np.float32)

    return {
        "x": x, "attn": attn, "enc": enc, "dls": dls, "die": die,
        "wrep": np.ascontiguousarray(np.broadcast_to(wcat[None, :], (P, 3 * D))),
        "idsf": np.ascontiguousarray(ids.astype(np.float32).reshape(4, P).T),
        "colsr": np.ascontiguousarray(np.broadcast_to(cols[None, :], (P, NSLOT))),
        "shift": np.ascontiguousarray(shift.reshape(NJC, P, NT2).transpose(1, 0, 2)),
        "biasr": np.full((P, 1), bias_total, np.float32),
        "identf": np.eye(P, dtype=np.float32),
        "identh": np.eye(P, dtype=np.float16),
        "idiv16": (np.eye(P, dtype=np.float32) / np.float32(H)),
    }


def kernel(**inputs) -> np.ndarray:
    from concourse.bass_utils import run_bass_kernel_spmd

    nc = _get_nc()
    in_maps = [_prep_core_inputs(inputs, c // 2, c % 2) for c in range(8)]
    res = run_bass_kernel_spmd(nc, in_maps, core_ids=list(range(8)))
    full = np.empty((B, T, V), np.float32)
    for c in range(8):
        b, th = c // 2, c % 2
        full[b, th * TH:(th + 1) * TH] = res.results[c]["out"]
    return full

